# revision 7
# baseline (speedup 1.0000x reference)
"""DiagBlockAttention Trainium2 kernel (Bass/Tile, 8 NeuronCores), v2.

Problem (hardcoded from spec nn_DiagBlockAttention):
  x[16, 3136, 768] -> qkv = x @ w_qkv -> 12 heads x 64
  block-local attention: 56x56 token grid, 4x4 spatial blocks (16 tokens),
  softmax over the 16 tokens of each block per head
  out = attn_out @ w_out + b_out

Sharding: data-parallel over batch, 2 batches per core.

v2 design (prior version: 989us; PE was instruction-rate bound: every
matmul pays ~170ns LDWEIGHTS + issue, so 4372 matmuls/core ~= runtime):
  * ALL data-layout work moved to the HOST: x is pre-permuted to block
    order AND pre-transposed to d-major AND pre-cast to bf16. The 672
    on-device PE transposes + psum copies of v1 are gone. The output is
    returned d-major bf16 and un-transposed on the host.
  * all matmuls bf16 (rel err ~5e-3 vs 2e-2 budget; fp8 fails: e4m3
    measured 4.3e-2 end-to-end in numpy sim).
  * scores per head-pair (hp) run as quadrant pairs: head 2hp on PE rows
    0:64, head 2hp+1 on rows 64:128 (disjoint tile_position -> the PE
    overlaps them, hiding the LDWEIGHTS floor).
  * PV emits o^T d-major directly: lhsT = v[s,64] so out = [64(d),112(tq)];
    the odd head targets psum partitions 64:128 (tile col 64), so a head
    PAIR packs one [128, 112] psum tile with zero garbage.
  * softmax denominators: Z_h[1,448] = ones[112,1]^T @ P_h (one matmul per
    head), reciprocal on DVE, then two K=1 broadcast matmuls expand
    rcp rows into a [128,448] per-pair scale; the oT write is a single
    fused tensor_mul (psum x scale -> bf16 oT) per (hp, g).
  * out projection d-major (stationary w_out tiles, moving oT), bias added
    as a per-partition tensor_scalar, store [128,448] bf16 d-major.
  * x DMA for chunk c+1 is issued before chunk c's compute (prefetch).
"""
import numpy as np
import ml_dtypes
from contextlib import ExitStack

import concourse.bass as bass
import concourse.mybir as mybir
import concourse.tile as tile
from concourse import bacc
from concourse.bass_utils import run_bass_kernel_spmd

# ---- problem constants ----
B, N, DIM = 16, 3136, 768
H, DH = 12, 64
J3 = 3 * H * DH              # 2304
SCALE = DH ** -0.5           # 0.125
NCORES = 8
B_LOC = B // NCORES          # 2
TOK = B_LOC * N              # 6272 tokens per core
CHUNK = 448
NCHUNK = TOK // CHUNK        # 14
NG = 4                       # groups per chunk
GT = 112                     # tokens per group (7 blocks x 16)
KT = DIM // 128              # 6 k-tiles
F32 = mybir.dt.float32
F32R = mybir.dt.float32r
BF16 = mybir.dt.bfloat16
NPBF16 = ml_dtypes.bfloat16

_CACHE = {}


def _build():
    nc = bacc.Bacc("TRN2", target_bir_lowering=False, debug=False)

    # x: HOST-prepared: block-ordered, d-major, bf16: x[kt, ki, t] = x[t, kt*128+ki]
    x_d = nc.dram_tensor("x", [KT, 128, TOK], BF16, kind="ExternalInput")
    wqkv_d = nc.dram_tensor("w_qkv", [DIM, J3], BF16, kind="ExternalInput")
    wout_d = nc.dram_tensor("w_out", [DIM, DIM], BF16, kind="ExternalInput")
    bout_d = nc.dram_tensor("b_out", [DIM], F32, kind="ExternalInput")
    # output d-major bf16; host un-transposes
    o_d = nc.dram_tensor("o", [KT, 128, TOK], BF16, kind="ExternalOutput")

    with tile.TileContext(nc) as tc, ExitStack() as ctx:
        const = ctx.enter_context(tc.tile_pool(name="const", bufs=1))
        wpool = ctx.enter_context(tc.tile_pool(name="w", bufs=1))
        xin = ctx.enter_context(tc.tile_pool(name="xin", bufs=2))
        qkp_ = ctx.enter_context(tc.tile_pool(name="qkt", bufs=2))
        vpool = ctx.enter_context(tc.tile_pool(name="vp", bufs=2))
        pmpool = ctx.enter_context(tc.tile_pool(name="pm", bufs=2))
        midp = ctx.enter_context(tc.tile_pool(name="mid", bufs=3))
        zrp = ctx.enter_context(tc.tile_pool(name="zr", bufs=3))
        bpool = ctx.enter_context(tc.tile_pool(name="bp", bufs=2))
        otp = ctx.enter_context(tc.tile_pool(name="ot", bufs=2))
        outp = ctx.enter_context(tc.tile_pool(name="outp", bufs=3))

        ps_big = ctx.enter_context(tc.tile_pool(name="ps_big", bufs=2, space="PSUM"))
        ps_s = ctx.enter_context(tc.tile_pool(name="ps_s", bufs=2, space="PSUM"))
        ps_pz = ctx.enter_context(tc.tile_pool(name="ps_pz", bufs=2, space="PSUM"))

        # ---- constants ----
        # 0/1 block-diag-16 mask x4 groups: on-block iff 0 <= p - 16*b7 <= 15
        mask = const.tile([GT, NG * GT], BF16)
        nc.gpsimd.memset(mask[:], 1.0)
        mask_v = mask[:].rearrange("p (g b7 ic) -> p g b7 ic", g=NG, b7=7)
        nc.gpsimd.affine_select(
            out=mask_v, in_=mask_v, compare_op=mybir.AluOpType.is_ge,
            fill=0.0, base=0, pattern=[[0, NG], [-16, 7], [0, 16]],
            channel_multiplier=1)
        nc.gpsimd.affine_select(
            out=mask_v, in_=mask_v, compare_op=mybir.AluOpType.is_ge,
            fill=0.0, base=15, pattern=[[0, NG], [16, 7], [0, 16]],
            channel_multiplier=-1)

        # Z stationaries: onesW[i] has ones in column i, zeros in the other,
        # so two accumulating matmuls build zp2 = [Z_h0; Z_h1] rows 0/1.
        onesW = const.tile([GT, 2, 2], BF16)
        nc.vector.memset(onesW[:], 0.0)
        nc.vector.memset(onesW[:, 0, 0:1], 1.0)
        nc.vector.memset(onesW[:, 1, 1:2], 1.0)
        # broadcast indicator: row 0 -> out partitions 0:64, row 1 -> 64:128
        # ind2f[p, col] = 1 iff col//64 == p, built via affine selects
        ind2f = const.tile([2, 128], F32)
        nc.gpsimd.memset(ind2f[:], 1.0)
        nc.gpsimd.affine_select(
            out=ind2f[:], in_=ind2f[:], compare_op=mybir.AluOpType.is_ge,
            fill=0.0, base=0, pattern=[[1, 128]], channel_multiplier=-64)
        nc.gpsimd.affine_select(
            out=ind2f[:], in_=ind2f[:], compare_op=mybir.AluOpType.is_ge,
            fill=0.0, base=63, pattern=[[-1, 128]], channel_multiplier=64)
        ind2 = const.tile([2, 128], F32R)
        with nc.allow_low_precision(reason="f32r == fp32 bits"):
            nc.vector.tensor_copy(ind2[:], ind2f[:])
        bias_sb = const.tile([128, KT], F32)
        nc.sync.dma_start(bias_sb[:],
                          bout_d.ap().rearrange("(ko ki) -> ki ko", ki=128))

        # ---- weights (bf16 straight from HBM) ----
        w_sb = wpool.tile([128, KT, J3], BF16)
        wo_sb = wpool.tile([128, KT, DIM], BF16)
        wq_src = wqkv_d.ap().rearrange("(ko ki) j -> ki ko j", ki=128)
        wo_src = wout_d.ap().rearrange("(ko ki) j -> ki ko j", ki=128)
        for kt in range(KT):
            nc.sync.dma_start(w_sb[:, kt, :], wq_src[:, kt, :])
            nc.sync.dma_start(wo_sb[:, kt, :], wo_src[:, kt, :])

        # ---- x prefetch helper ----
        def load_x(c):
            xt = xin.tile([128, KT, CHUNK], BF16, tag="xT", name=f"xT{c % 2}")
            for kt in range(KT):
                nc.sync.dma_start(xt[:, kt, :],
                                  x_d.ap()[kt, :, c * CHUNK:(c + 1) * CHUNK])
            return xt

        xt_next = load_x(0)
        for c in range(NCHUNK):
            xT = xt_next
            if c + 1 < NCHUNK:
                xt_next = load_x(c + 1)

            # ---- qk projection, d-major [j-tile 128, 448] ----
            qk = qkp_.tile([128, 12, CHUNK], BF16, tag="qk")
            for jt in range(12):
                qkp = ps_big.tile([128, CHUNK], F32, tag="big", name="qkp")
                for kt in range(KT):
                    nc.tensor.matmul(
                        qkp[:], w_sb[:, kt, jt * 128:(jt + 1) * 128],
                        xT[:, kt, :], start=(kt == 0), stop=(kt == KT - 1))
                nc.vector.tensor_copy(qk[:, jt, :], qkp[:])

            # ---- v projection, token-major [112, 384] x2 per group ----
            v2 = vpool.tile([GT, NG, H, DH], BF16, tag="v2")
            for g in range(NG):
                gs = slice(g * GT, (g + 1) * GT)
                for half in range(2):
                    vp = ps_big.tile([GT, CHUNK], F32, tag="big", name="vp")
                    for kt in range(KT):
                        nc.tensor.matmul(
                            vp[:, 0:384], xT[:, kt, gs],
                            w_sb[:, kt, 1536 + half * 384:1536 + (half + 1) * 384],
                            start=(kt == 0), stop=(kt == KT - 1))
                    nc.scalar.copy(
                        v2[:, g, half * 6:(half + 1) * 6, :],
                        vp[:, 0:384].rearrange("p (h d) -> p h d", d=DH))

            # ---- scores (quadrant pairs) + exp + mask ----
            pm = pmpool.tile([GT, H, NG * GT], BF16, tag="pm")
            for hp in range(6):
                sp0 = ps_s.tile([GT, NG * GT], F32, tag="sp", name="sp0")
                sp1 = ps_s.tile([GT, NG * GT], F32, tag="sp", name="sp1")
                for g in range(NG):
                    gs = slice(g * GT, (g + 1) * GT)
                    nc.tensor.matmul(sp0[:, gs], qk[0:64, 6 + hp, gs],
                                     qk[0:64, hp, gs], start=True, stop=True)
                    nc.tensor.matmul(sp1[:, gs], qk[64:128, 6 + hp, gs],
                                     qk[64:128, hp, gs], start=True, stop=True)
                for i, sp in enumerate((sp0, sp1)):
                    pme = midp.tile([GT, NG * GT], BF16, tag="pme",
                                    name=f"pme{i}")
                    nc.scalar.activation(pme[:], sp[:],
                                         mybir.ActivationFunctionType.Exp,
                                         scale=SCALE)
                    nc.vector.tensor_mul(pm[:, 2 * hp + i, :], pme[:], mask[:])

            # ---- Z, reciprocal, broadcast scales ----
            bpssb = []
            for hp in range(6):
                zp2 = ps_pz.tile([2, NG * GT], F32, tag="zp", name="zp")
                nc.tensor.matmul(zp2[:], onesW[:, 0, :], pm[:, 2 * hp, :],
                                 start=True, stop=False)
                nc.tensor.matmul(zp2[:], onesW[:, 1, :], pm[:, 2 * hp + 1, :],
                                 start=False, stop=True)
                zr2 = zrp.tile([2, NG * GT], F32R, tag="zr", name="zr")
                # f32r output is bitwise fp32 - no precision loss
                with nc.allow_low_precision(reason="f32r == fp32 bits"):
                    nc.vector.reciprocal(zr2[:], zp2[:])
                bps = ps_big.tile([128, CHUNK], F32, tag="big", name="bps")
                nc.tensor.matmul(bps[:], ind2[:], zr2[:],
                                 start=True, stop=True)
                bsb = bpool.tile([128, CHUNK], F32, tag="bpssb", name="bsb")
                nc.scalar.copy(bsb[:], bps[:])
                bpssb.append(bsb)

            # ---- PV (quadrant pairs, d-major out) + fused normalize ----
            oT = otp.tile([128, KT, CHUNK], BF16, tag="oT")
            for hp in range(6):
                for g in range(NG):
                    gs = slice(g * GT, (g + 1) * GT)
                    pvp = ps_pz.tile([128, GT], F32, tag="pvp", name="pvp")
                    nc.tensor.matmul(pvp[0:64, :], v2[:, g, 2 * hp, :],
                                     pm[:, 2 * hp, gs], start=True, stop=True)
                    nc.tensor.matmul(pvp[64:128, :], v2[:, g, 2 * hp + 1, :],
                                     pm[:, 2 * hp + 1, gs],
                                     start=True, stop=True)
                    nc.vector.tensor_mul(oT[:, hp, gs], pvp[:],
                                         bpssb[hp][:, gs])

            # ---- out projection d-major + bias + store ----
            for mt in range(KT):
                ops = ps_big.tile([128, CHUNK], F32, tag="big", name="ops")
                for kt in range(KT):
                    nc.tensor.matmul(
                        ops[:], wo_sb[:, kt, mt * 128:(mt + 1) * 128],
                        oT[:, kt, :], start=(kt == 0), stop=(kt == KT - 1))
                out_sb = outp.tile([128, CHUNK], BF16, tag="out_sb")
                nc.vector.tensor_scalar_add(out_sb[:], ops[:],
                                            bias_sb[:, mt:mt + 1])
                nc.sync.dma_start(o_d.ap()[mt, :, c * CHUNK:(c + 1) * CHUNK],
                                  out_sb[:])

    nc.compile()
    return nc


def _to_blocks_host(x):
    """[B_sub, 3136, d] raster -> [B_sub, ch, g, (b7 ir ic), d] block order."""
    b, n, d = x.shape
    # n = (ch, br, ir, h2, b7, ic) with sizes (7, 2, 4, 2, 7, 4)
    x = x.reshape(b, 7, 2, 4, 2, 7, 4, d)
    x = x.transpose(0, 1, 2, 4, 5, 3, 6, 7)   # b ch br h2 b7 ir ic d
    return np.ascontiguousarray(x.reshape(b, 7, NG, GT, d))


def _from_blocks_host(o):
    """[B_sub, ch, g, t, d] block order -> [B_sub, 3136, d]."""
    b = o.shape[0]
    o = o.reshape(b, 7, 2, 2, 7, 4, 4, DIM)   # b ch br h2 b7 ir ic d
    o = o.transpose(0, 1, 2, 5, 3, 4, 6, 7)   # b ch br ir h2 b7 ic d
    return np.ascontiguousarray(o.reshape(b, N, DIM))


def _prep_x_core(x_core):
    """[B_LOC, 3136, 768] fp32 -> [KT, 128, TOK] bf16 d-major block order."""
    xb = _to_blocks_host(x_core)                     # [2, 7, 4, 112, 768]
    xt = xb.reshape(TOK, DIM).T                      # [768, 6272]
    return np.ascontiguousarray(xt.reshape(KT, 128, TOK)).astype(NPBF16)


def _unprep_o_core(o6):
    """[KT, 128, TOK] bf16 d-major -> [B_LOC, 3136, 768] fp32."""
    o = np.asarray(o6, dtype=np.float32).reshape(DIM, TOK).T  # [6272, 768]
    o = np.ascontiguousarray(o).reshape(B_LOC, 7, NG, GT, DIM)
    return _from_blocks_host(o)


def make_in_maps(x, w_qkv, w_out, b_out):
    x = np.ascontiguousarray(x, dtype=np.float32)
    wq = np.ascontiguousarray(w_qkv, dtype=np.float32).astype(NPBF16)
    wo = np.ascontiguousarray(w_out, dtype=np.float32).astype(NPBF16)
    bo = np.ascontiguousarray(b_out, dtype=np.float32)
    return [
        {"x": _prep_x_core(x[c * B_LOC:(c + 1) * B_LOC]),
         "w_qkv": wq, "w_out": wo, "b_out": bo}
        for c in range(NCORES)
    ]


def kernel(x, w_qkv, w_out, b_out):
    if "nc" not in _CACHE:
        _CACHE["nc"] = _build()
    nc = _CACHE["nc"]

    in_maps = make_in_maps(x, w_qkv, w_out, b_out)
    res = run_bass_kernel_spmd(nc, in_maps, core_ids=list(range(NCORES)))
    out = np.concatenate(
        [_unprep_o_core(res.results[c]["o"]) for c in range(NCORES)], axis=0)
    return out.astype(np.float32)


# revision 13
# speedup vs baseline: 1.4413x; 1.4413x over previous
"""DiagBlockAttention Trainium2 kernel (Bass/Tile, 8 NeuronCores), v2.

Problem (hardcoded from spec nn_DiagBlockAttention):
  x[16, 3136, 768] -> qkv = x @ w_qkv -> 12 heads x 64
  block-local attention: 56x56 token grid, 4x4 spatial blocks (16 tokens),
  softmax over the 16 tokens of each block per head
  out = attn_out @ w_out + b_out

Sharding: data-parallel over batch, 2 batches per core.

v2 design (prior version: 989us; PE was instruction-rate bound: every
matmul pays ~170ns LDWEIGHTS + issue, so 4372 matmuls/core ~= runtime):
  * ALL data-layout work moved to the HOST: x is pre-permuted to block
    order AND pre-transposed to d-major AND pre-cast to bf16. The 672
    on-device PE transposes + psum copies of v1 are gone. The output is
    returned d-major bf16 and un-transposed on the host.
  * all matmuls bf16 (rel err ~5e-3 vs 2e-2 budget; fp8 fails: e4m3
    measured 4.3e-2 end-to-end in numpy sim).
  * scores per head-pair (hp) run as quadrant pairs: head 2hp on PE rows
    0:64, head 2hp+1 on rows 64:128 (disjoint tile_position -> the PE
    overlaps them, hiding the LDWEIGHTS floor).
  * PV emits o^T d-major directly: lhsT = v[s,64] so out = [64(d),112(tq)];
    the odd head targets psum partitions 64:128 (tile col 64), so a head
    PAIR packs one [128, 112] psum tile with zero garbage.
  * softmax denominators: Z_h[1,448] = ones[112,1]^T @ P_h (one matmul per
    head), reciprocal on DVE, then two K=1 broadcast matmuls expand
    rcp rows into a [128,448] per-pair scale; the oT write is a single
    fused tensor_mul (psum x scale -> bf16 oT) per (hp, g).
  * out projection d-major (stationary w_out tiles, moving oT), bias added
    as a per-partition tensor_scalar, store [128,448] bf16 d-major.
  * x DMA for chunk c+1 is issued before chunk c's compute (prefetch).
"""
import numpy as np
import ml_dtypes
from contextlib import ExitStack

import concourse.bass as bass
import concourse.mybir as mybir
import concourse.tile as tile
from concourse import bacc
from concourse.bass_utils import run_bass_kernel_spmd

# ---- problem constants ----
B, N, DIM = 16, 3136, 768
H, DH = 12, 64
J3 = 3 * H * DH              # 2304
SCALE = DH ** -0.5           # 0.125
NCORES = 8
B_LOC = B // NCORES          # 2
TOK = B_LOC * N              # 6272 tokens per core
CHUNK = 448
NCHUNK = TOK // CHUNK        # 14
NG = 4                       # groups per chunk
GT = 112                     # tokens per group (7 blocks x 16)
KT = DIM // 128              # 6 k-tiles
F32 = mybir.dt.float32
F32R = mybir.dt.float32r
BF16 = mybir.dt.bfloat16
NPBF16 = ml_dtypes.bfloat16

_CACHE = {}


def _build():
    nc = bacc.Bacc("TRN2", target_bir_lowering=False, debug=False)

    # x: HOST-prepared: block-ordered, d-major, bf16: x[kt, ki, t] = x[t, kt*128+ki]
    x_d = nc.dram_tensor("x", [KT, 128, TOK], BF16, kind="ExternalInput")
    wqkv_d = nc.dram_tensor("w_qkv", [DIM, J3], BF16, kind="ExternalInput")
    wout_d = nc.dram_tensor("w_out", [DIM, DIM], BF16, kind="ExternalInput")
    bout_d = nc.dram_tensor("b_out", [DIM], F32, kind="ExternalInput")
    # output d-major bf16; host un-transposes
    o_d = nc.dram_tensor("o", [KT, 128, TOK], BF16, kind="ExternalOutput")

    with tile.TileContext(nc) as tc, ExitStack() as ctx:
        const = ctx.enter_context(tc.tile_pool(name="const", bufs=1))
        wpool = ctx.enter_context(tc.tile_pool(name="w", bufs=1))
        xin = ctx.enter_context(tc.tile_pool(name="xin", bufs=2))
        qkp_ = ctx.enter_context(tc.tile_pool(name="qkt", bufs=2))
        vpool = ctx.enter_context(tc.tile_pool(name="vp", bufs=2))
        pmpool = ctx.enter_context(tc.tile_pool(name="pm", bufs=2))
        midp = ctx.enter_context(tc.tile_pool(name="mid", bufs=3))
        zrp = ctx.enter_context(tc.tile_pool(name="zr", bufs=3))
        bpool = ctx.enter_context(tc.tile_pool(name="bp", bufs=2))
        otp = ctx.enter_context(tc.tile_pool(name="ot", bufs=2))
        outp = ctx.enter_context(tc.tile_pool(name="outp", bufs=3))

        ps_big = ctx.enter_context(tc.tile_pool(name="ps_big", bufs=2, space="PSUM"))
        ps_s = ctx.enter_context(tc.tile_pool(name="ps_s", bufs=3, space="PSUM"))
        ps_z = ctx.enter_context(tc.tile_pool(name="ps_z", bufs=1, space="PSUM"))
        ps_pv = ctx.enter_context(tc.tile_pool(name="ps_pv", bufs=2, space="PSUM"))

        # ---- constants ----
        # 0/1 block-diag-16 mask x4 groups: on-block iff 0 <= p - 16*b7 <= 15
        mask = const.tile([GT, NG * GT], BF16)
        nc.gpsimd.memset(mask[:], 1.0)
        mask_v = mask[:].rearrange("p (g b7 ic) -> p g b7 ic", g=NG, b7=7)
        nc.gpsimd.affine_select(
            out=mask_v, in_=mask_v, compare_op=mybir.AluOpType.is_ge,
            fill=0.0, base=0, pattern=[[0, NG], [-16, 7], [0, 16]],
            channel_multiplier=1)
        nc.gpsimd.affine_select(
            out=mask_v, in_=mask_v, compare_op=mybir.AluOpType.is_ge,
            fill=0.0, base=15, pattern=[[0, NG], [16, 7], [0, 16]],
            channel_multiplier=-1)

        # Z stationaries: onesW[i] has ones in column i, zeros in the other,
        # so two accumulating matmuls build zp2 = [Z_h0; Z_h1] rows 0/1.
        onesW = const.tile([GT, 2, 2], BF16)
        nc.vector.memset(onesW[:], 0.0)
        nc.vector.memset(onesW[:, 0, 0:1], 1.0)
        nc.vector.memset(onesW[:, 1, 1:2], 1.0)
        # broadcast indicator: row 0 -> out partitions 0:64, row 1 -> 64:128
        # ind2f[p, col] = 1 iff col//64 == p, built via affine selects
        ind2f = const.tile([2, 128], F32)
        nc.gpsimd.memset(ind2f[:], 1.0)
        nc.gpsimd.affine_select(
            out=ind2f[:], in_=ind2f[:], compare_op=mybir.AluOpType.is_ge,
            fill=0.0, base=0, pattern=[[1, 128]], channel_multiplier=-64)
        nc.gpsimd.affine_select(
            out=ind2f[:], in_=ind2f[:], compare_op=mybir.AluOpType.is_ge,
            fill=0.0, base=63, pattern=[[-1, 128]], channel_multiplier=64)
        ind2 = const.tile([2, 128], F32R)
        with nc.allow_low_precision(reason="f32r == fp32 bits"):
            nc.vector.tensor_copy(ind2[:], ind2f[:])
        bias_sb = const.tile([128, KT], F32)
        nc.sync.dma_start(bias_sb[:],
                          bout_d.ap().rearrange("(ko ki) -> ki ko", ki=128))

        # ---- weights (bf16 straight from HBM) ----
        w_sb = wpool.tile([128, KT, J3], BF16)
        wo_sb = wpool.tile([128, KT, DIM], BF16)
        wq_src = wqkv_d.ap().rearrange("(ko ki) j -> ki ko j", ki=128)
        wo_src = wout_d.ap().rearrange("(ko ki) j -> ki ko j", ki=128)
        for kt in range(KT):
            nc.sync.dma_start(w_sb[:, kt, :], wq_src[:, kt, :])
            nc.sync.dma_start(wo_sb[:, kt, :], wo_src[:, kt, :])

        # ---- x prefetch helper ----
        def load_x(c):
            xt = xin.tile([128, KT, CHUNK], BF16, tag="xT", name=f"xT{c % 2}")
            for kt in range(KT):
                nc.sync.dma_start(xt[:, kt, :],
                                  x_d.ap()[kt, :, c * CHUNK:(c + 1) * CHUNK])
            return xt

        xt_next = load_x(0)
        for c in range(NCHUNK):
            xT = xt_next
            if c + 1 < NCHUNK:
                xt_next = load_x(c + 1)

            # ---- qk projection, d-major [j-tile 128, 448] ----
            qk = qkp_.tile([128, 12, CHUNK], BF16, tag="qk")
            for jt in range(12):
                qkp = ps_big.tile([128, CHUNK], F32, tag="big", name="qkp")
                for kt in range(KT):
                    nc.tensor.matmul(
                        qkp[:], w_sb[:, kt, jt * 128:(jt + 1) * 128],
                        xT[:, kt, :], start=(kt == 0), stop=(kt == KT - 1))
                nc.vector.tensor_copy(qk[:, jt, :], qkp[:])

            # ---- v projection, token-major [112, 384] x2 per group ----
            v2 = vpool.tile([GT, NG, H, DH], BF16, tag="v2")
            for g in range(NG):
                gs = slice(g * GT, (g + 1) * GT)
                for half in range(2):
                    vp = ps_big.tile([GT, CHUNK], F32, tag="big", name="vp")
                    for kt in range(KT):
                        nc.tensor.matmul(
                            vp[:, 0:384], xT[:, kt, gs],
                            w_sb[:, kt, 1536 + half * 384:1536 + (half + 1) * 384],
                            start=(kt == 0), stop=(kt == KT - 1))
                    nc.scalar.copy(
                        v2[:, g, half * 6:(half + 1) * 6, :],
                        vp[:, 0:384].rearrange("p (h d) -> p h d", d=DH))

            # ---- scores (quadrant pairs) + exp + mask ----
            pm = pmpool.tile([GT, H, NG * GT], BF16, tag="pm")
            for hp in range(6):
                sp0 = ps_s.tile([GT, NG * GT], F32, tag="sp", name="sp0")
                sp1 = ps_s.tile([GT, NG * GT], F32, tag="sp", name="sp1")
                for g in range(NG):
                    gs = slice(g * GT, (g + 1) * GT)
                    nc.tensor.matmul(sp0[:, gs], qk[0:64, 6 + hp, gs],
                                     qk[0:64, hp, gs], start=True, stop=True)
                    nc.tensor.matmul(sp1[:, gs], qk[64:128, 6 + hp, gs],
                                     qk[64:128, hp, gs], start=True, stop=True)
                for i, sp in enumerate((sp0, sp1)):
                    pme = midp.tile([GT, NG * GT], BF16, tag="pme",
                                    name=f"pme{i}")
                    nc.scalar.activation(pme[:], sp[:],
                                         mybir.ActivationFunctionType.Exp,
                                         scale=SCALE)
                    nc.vector.tensor_mul(pm[:, 2 * hp + i, :], pme[:], mask[:])

            # ---- Z, reciprocal, broadcast scale per head-pair ----
            bpss = []
            for hp in range(6):
                zp2 = ps_z.tile([2, NG * GT], F32, tag="zp", name="zp")
                nc.tensor.matmul(zp2[:], onesW[:, 0, :], pm[:, 2 * hp, :],
                                 start=True, stop=False)
                nc.tensor.matmul(zp2[:], onesW[:, 1, :], pm[:, 2 * hp + 1, :],
                                 start=False, stop=True)
                zr2 = zrp.tile([2, NG * GT], F32, tag="zr", name="zrf")
                nc.vector.reciprocal_approx_fast(zr2[:], zp2[:])
                zr2r = zrp.tile([2, NG * GT], F32R, tag="zrr", name="zrr")
                with nc.allow_low_precision(reason="f32r rounding for PE"):
                    nc.vector.tensor_copy(zr2r[:], zr2[:])
                bps = ps_big.tile([128, CHUNK], F32, tag="big", name="bps")
                nc.tensor.matmul(bps[:], ind2[:], zr2r[:],
                                 start=True, stop=True)
                bsb = bpool.tile([128, CHUNK], F32, tag="bpssb", name="bsb")
                nc.scalar.copy(bsb[:], bps[:])
                bpss.append(bsb)

            # ---- PV (quadrant pairs, d-major out) + fused normalize ----
            oT = otp.tile([128, KT, CHUNK], BF16, tag="oT")
            for hp in range(6):
                for g in range(NG):
                    gs = slice(g * GT, (g + 1) * GT)
                    pvp = ps_pv.tile([128, GT], F32, tag="pvp", name="pvp")
                    nc.tensor.matmul(pvp[0:64, :], v2[:, g, 2 * hp, :],
                                     pm[:, 2 * hp, gs], start=True, stop=True)
                    nc.tensor.matmul(pvp[64:128, :], v2[:, g, 2 * hp + 1, :],
                                     pm[:, 2 * hp + 1, gs],
                                     start=True, stop=True)
                    nc.vector.tensor_mul(oT[:, hp, gs], pvp[:],
                                         bpss[hp][:, gs])

            # ---- out projection d-major + bias + store ----
            for mt in range(KT):
                ops = ps_big.tile([128, CHUNK], F32, tag="big", name="ops")
                for kt in range(KT):
                    nc.tensor.matmul(
                        ops[:], wo_sb[:, kt, mt * 128:(mt + 1) * 128],
                        oT[:, kt, :], start=(kt == 0), stop=(kt == KT - 1))
                out_sb = outp.tile([128, CHUNK], BF16, tag="out_sb")
                nc.vector.tensor_scalar_add(out_sb[:], ops[:],
                                            bias_sb[:, mt:mt + 1])
                nc.sync.dma_start(o_d.ap()[mt, :, c * CHUNK:(c + 1) * CHUNK],
                                  out_sb[:])

    nc.compile()
    return nc


def _to_blocks_host(x):
    """[B_sub, 3136, d] raster -> [B_sub, ch, g, (b7 ir ic), d] block order."""
    b, n, d = x.shape
    # n = (ch, br, ir, h2, b7, ic) with sizes (7, 2, 4, 2, 7, 4)
    x = x.reshape(b, 7, 2, 4, 2, 7, 4, d)
    x = x.transpose(0, 1, 2, 4, 5, 3, 6, 7)   # b ch br h2 b7 ir ic d
    return np.ascontiguousarray(x.reshape(b, 7, NG, GT, d))


def _from_blocks_host(o):
    """[B_sub, ch, g, t, d] block order -> [B_sub, 3136, d]."""
    b = o.shape[0]
    o = o.reshape(b, 7, 2, 2, 7, 4, 4, DIM)   # b ch br h2 b7 ir ic d
    o = o.transpose(0, 1, 2, 5, 3, 4, 6, 7)   # b ch br ir h2 b7 ic d
    return np.ascontiguousarray(o.reshape(b, N, DIM))


def _prep_x_core(x_core):
    """[B_LOC, 3136, 768] fp32 -> [KT, 128, TOK] bf16 d-major block order."""
    xb = _to_blocks_host(x_core)                     # [2, 7, 4, 112, 768]
    xt = xb.reshape(TOK, DIM).T                      # [768, 6272]
    return np.ascontiguousarray(xt.reshape(KT, 128, TOK)).astype(NPBF16)


def _unprep_o_core(o6):
    """[KT, 128, TOK] bf16 d-major -> [B_LOC, 3136, 768] fp32."""
    o = np.asarray(o6, dtype=np.float32).reshape(DIM, TOK).T  # [6272, 768]
    o = np.ascontiguousarray(o).reshape(B_LOC, 7, NG, GT, DIM)
    return _from_blocks_host(o)


def make_in_maps(x, w_qkv, w_out, b_out):
    x = np.ascontiguousarray(x, dtype=np.float32)
    wq = np.ascontiguousarray(w_qkv, dtype=np.float32).astype(NPBF16)
    wo = np.ascontiguousarray(w_out, dtype=np.float32).astype(NPBF16)
    bo = np.ascontiguousarray(b_out, dtype=np.float32)
    return [
        {"x": _prep_x_core(x[c * B_LOC:(c + 1) * B_LOC]),
         "w_qkv": wq, "w_out": wo, "b_out": bo}
        for c in range(NCORES)
    ]


def kernel(x, w_qkv, w_out, b_out):
    if "nc" not in _CACHE:
        _CACHE["nc"] = _build()
    nc = _CACHE["nc"]

    in_maps = make_in_maps(x, w_qkv, w_out, b_out)
    res = run_bass_kernel_spmd(nc, in_maps, core_ids=list(range(NCORES)))
    out = np.concatenate(
        [_unprep_o_core(res.results[c]["o"]) for c in range(NCORES)], axis=0)
    return out.astype(np.float32)


# revision 15
# speedup vs baseline: 1.4742x; 1.0228x over previous
"""DiagBlockAttention Trainium2 kernel (Bass/Tile, 8 NeuronCores), v2.

Problem (hardcoded from spec nn_DiagBlockAttention):
  x[16, 3136, 768] -> qkv = x @ w_qkv -> 12 heads x 64
  block-local attention: 56x56 token grid, 4x4 spatial blocks (16 tokens),
  softmax over the 16 tokens of each block per head
  out = attn_out @ w_out + b_out

Sharding: data-parallel over batch, 2 batches per core.

v2 design (prior version: 989us; PE was instruction-rate bound: every
matmul pays ~170ns LDWEIGHTS + issue, so 4372 matmuls/core ~= runtime):
  * ALL data-layout work moved to the HOST: x is pre-permuted to block
    order AND pre-transposed to d-major AND pre-cast to bf16. The 672
    on-device PE transposes + psum copies of v1 are gone. The output is
    returned d-major bf16 and un-transposed on the host.
  * all matmuls bf16 (rel err ~5e-3 vs 2e-2 budget; fp8 fails: e4m3
    measured 4.3e-2 end-to-end in numpy sim).
  * scores per head-pair (hp) run as quadrant pairs: head 2hp on PE rows
    0:64, head 2hp+1 on rows 64:128 (disjoint tile_position -> the PE
    overlaps them, hiding the LDWEIGHTS floor).
  * PV emits o^T d-major directly: lhsT = v[s,64] so out = [64(d),112(tq)];
    the odd head targets psum partitions 64:128 (tile col 64), so a head
    PAIR packs one [128, 112] psum tile with zero garbage.
  * softmax denominators: Z_h[1,448] = ones[112,1]^T @ P_h (one matmul per
    head), reciprocal on DVE, then two K=1 broadcast matmuls expand
    rcp rows into a [128,448] per-pair scale; the oT write is a single
    fused tensor_mul (psum x scale -> bf16 oT) per (hp, g).
  * out projection d-major (stationary w_out tiles, moving oT), bias added
    as a per-partition tensor_scalar, store [128,448] bf16 d-major.
  * x DMA for chunk c+1 is issued before chunk c's compute (prefetch).
"""
import numpy as np
import ml_dtypes
from contextlib import ExitStack

import concourse.bass as bass
import concourse.mybir as mybir
import concourse.tile as tile
from concourse import bacc
from concourse.bass_utils import run_bass_kernel_spmd

# ---- problem constants ----
B, N, DIM = 16, 3136, 768
H, DH = 12, 64
J3 = 3 * H * DH              # 2304
SCALE = DH ** -0.5           # 0.125
NCORES = 8
B_LOC = B // NCORES          # 2
TOK = B_LOC * N              # 6272 tokens per core
CHUNK = 448
NCHUNK = TOK // CHUNK        # 14
NG = 4                       # groups per chunk
GT = 112                     # tokens per group (7 blocks x 16)
KT = DIM // 128              # 6 k-tiles
F32 = mybir.dt.float32
F32R = mybir.dt.float32r
BF16 = mybir.dt.bfloat16
NPBF16 = ml_dtypes.bfloat16

_CACHE = {}


def _build():
    nc = bacc.Bacc("TRN2", target_bir_lowering=False, debug=False)

    # x: HOST-prepared: block-ordered, d-major, bf16: x[kt, ki, t] = x[t, kt*128+ki]
    x_d = nc.dram_tensor("x", [KT, 128, TOK], BF16, kind="ExternalInput")
    wqkv_d = nc.dram_tensor("w_qkv", [DIM, J3], BF16, kind="ExternalInput")
    wout_d = nc.dram_tensor("w_out", [DIM, DIM], BF16, kind="ExternalInput")
    bout_d = nc.dram_tensor("b_out", [DIM], F32, kind="ExternalInput")
    # output d-major bf16; host un-transposes
    o_d = nc.dram_tensor("o", [KT, 128, TOK], BF16, kind="ExternalOutput")

    with tile.TileContext(nc) as tc, ExitStack() as ctx:
        const = ctx.enter_context(tc.tile_pool(name="const", bufs=1))
        wpool = ctx.enter_context(tc.tile_pool(name="w", bufs=1))
        xin = ctx.enter_context(tc.tile_pool(name="xin", bufs=2))
        qkp_ = ctx.enter_context(tc.tile_pool(name="qkt", bufs=2))
        vpool = ctx.enter_context(tc.tile_pool(name="vp", bufs=2))
        pmpool = ctx.enter_context(tc.tile_pool(name="pm", bufs=2))
        midp = ctx.enter_context(tc.tile_pool(name="mid", bufs=3))
        zrp = ctx.enter_context(tc.tile_pool(name="zr", bufs=3))
        bpool = ctx.enter_context(tc.tile_pool(name="bp", bufs=2))
        otp = ctx.enter_context(tc.tile_pool(name="ot", bufs=2))
        outp = ctx.enter_context(tc.tile_pool(name="outp", bufs=3))

        ps_big = ctx.enter_context(tc.tile_pool(name="ps_big", bufs=2, space="PSUM"))
        ps_s = ctx.enter_context(tc.tile_pool(name="ps_s", bufs=2, space="PSUM"))
        ps_z = ctx.enter_context(tc.tile_pool(name="ps_z", bufs=2, space="PSUM"))
        ps_pv = ctx.enter_context(tc.tile_pool(name="ps_pv", bufs=2, space="PSUM"))

        # ---- constants ----
        # 0/1 block-diag-16 mask x4 groups: on-block iff 0 <= p - 16*b7 <= 15
        mask = const.tile([GT, NG * GT], BF16)
        nc.gpsimd.memset(mask[:], 1.0)
        mask_v = mask[:].rearrange("p (g b7 ic) -> p g b7 ic", g=NG, b7=7)
        nc.gpsimd.affine_select(
            out=mask_v, in_=mask_v, compare_op=mybir.AluOpType.is_ge,
            fill=0.0, base=0, pattern=[[0, NG], [-16, 7], [0, 16]],
            channel_multiplier=1)
        nc.gpsimd.affine_select(
            out=mask_v, in_=mask_v, compare_op=mybir.AluOpType.is_ge,
            fill=0.0, base=15, pattern=[[0, NG], [16, 7], [0, 16]],
            channel_multiplier=-1)

        # Z stationaries: onesW[i] has ones in column i, zeros in the other,
        # so two accumulating matmuls build zp2 = [Z_h0; Z_h1] rows 0/1.
        onesW = const.tile([GT, 2, 2], BF16)
        nc.vector.memset(onesW[:], 0.0)
        nc.vector.memset(onesW[:, 0, 0:1], 1.0)
        nc.vector.memset(onesW[:, 1, 1:2], 1.0)
        # broadcast indicator: row 0 -> out partitions 0:64, row 1 -> 64:128
        # ind2f[p, col] = 1 iff col//64 == p, built via affine selects
        ind2f = const.tile([2, 128], F32)
        nc.gpsimd.memset(ind2f[:], 1.0)
        nc.gpsimd.affine_select(
            out=ind2f[:], in_=ind2f[:], compare_op=mybir.AluOpType.is_ge,
            fill=0.0, base=0, pattern=[[1, 128]], channel_multiplier=-64)
        nc.gpsimd.affine_select(
            out=ind2f[:], in_=ind2f[:], compare_op=mybir.AluOpType.is_ge,
            fill=0.0, base=63, pattern=[[-1, 128]], channel_multiplier=64)
        ind2 = const.tile([2, 128], F32R)
        with nc.allow_low_precision(reason="f32r == fp32 bits"):
            nc.vector.tensor_copy(ind2[:], ind2f[:])
        bias_sb = const.tile([128, KT], F32)
        nc.sync.dma_start(bias_sb[:],
                          bout_d.ap().rearrange("(ko ki) -> ki ko", ki=128))

        # ---- weights (bf16 straight from HBM) ----
        w_sb = wpool.tile([128, KT, J3], BF16)
        wo_sb = wpool.tile([128, KT, DIM], BF16)
        wq_src = wqkv_d.ap().rearrange("(ko ki) j -> ki ko j", ki=128)
        wo_src = wout_d.ap().rearrange("(ko ki) j -> ki ko j", ki=128)
        for kt in range(KT):
            nc.sync.dma_start(w_sb[:, kt, :], wq_src[:, kt, :])
            nc.sync.dma_start(wo_sb[:, kt, :], wo_src[:, kt, :])

        # ---- x prefetch helper ----
        def load_x(c):
            xt = xin.tile([128, KT, CHUNK], BF16, tag="xT", name=f"xT{c % 2}")
            for kt in range(KT):
                nc.sync.dma_start(xt[:, kt, :],
                                  x_d.ap()[kt, :, c * CHUNK:(c + 1) * CHUNK])
            return xt

        def emit_out_proj(c, oT):
            # ---- out projection d-major + bias + store (chunk c) ----
            for mt in range(KT):
                ops = ps_big.tile([128, CHUNK], F32, tag="big", name="ops")
                for kt in range(KT):
                    nc.tensor.matmul(
                        ops[:], wo_sb[:, kt, mt * 128:(mt + 1) * 128],
                        oT[:, kt, :], start=(kt == 0), stop=(kt == KT - 1))
                out_sb = outp.tile([128, CHUNK], BF16, tag="out_sb")
                nc.vector.tensor_scalar_add(out_sb[:], ops[:],
                                            bias_sb[:, mt:mt + 1])
                nc.sync.dma_start(o_d.ap()[mt, :, c * CHUNK:(c + 1) * CHUNK],
                                  out_sb[:])

        xt_next = load_x(0)
        oT_prev = None
        for c in range(NCHUNK):
            xT = xt_next
            if c + 1 < NCHUNK:
                xt_next = load_x(c + 1)

            # ---- qk projection, d-major [j-tile 128, 448] ----
            qk = qkp_.tile([128, 12, CHUNK], BF16, tag="qk")
            for jt in range(12):
                qkp = ps_big.tile([128, CHUNK], F32, tag="big", name="qkp")
                for kt in range(KT):
                    nc.tensor.matmul(
                        qkp[:], w_sb[:, kt, jt * 128:(jt + 1) * 128],
                        xT[:, kt, :], start=(kt == 0), stop=(kt == KT - 1))
                nc.vector.tensor_copy(qk[:, jt, :], qkp[:])

            if oT_prev is not None:
                emit_out_proj(c - 1, oT_prev)

            # ---- v projection, token-major [112, 384] x2 per group ----
            v2 = vpool.tile([GT, NG, H, DH], BF16, tag="v2")
            for g in range(NG):
                gs = slice(g * GT, (g + 1) * GT)
                for half in range(2):
                    vp = ps_big.tile([GT, CHUNK], F32, tag="big", name="vp")
                    for kt in range(KT):
                        nc.tensor.matmul(
                            vp[:, 0:384], xT[:, kt, gs],
                            w_sb[:, kt, 1536 + half * 384:1536 + (half + 1) * 384],
                            start=(kt == 0), stop=(kt == KT - 1))
                    nc.scalar.copy(
                        v2[:, g, half * 6:(half + 1) * 6, :],
                        vp[:, 0:384].rearrange("p (h d) -> p h d", d=DH))

            # ---- scores (quadrant pairs) + exp + mask ----
            pm = pmpool.tile([GT, H, NG * GT], BF16, tag="pm")
            for hp in range(6):
                sp0 = ps_s.tile([GT, NG * GT], F32, tag="sp", name="sp0")
                sp1 = ps_s.tile([GT, NG * GT], F32, tag="sp", name="sp1")
                for g in range(NG):
                    gs = slice(g * GT, (g + 1) * GT)
                    nc.tensor.matmul(sp0[:, gs], qk[0:64, 6 + hp, gs],
                                     qk[0:64, hp, gs], start=True, stop=True)
                    nc.tensor.matmul(sp1[:, gs], qk[64:128, 6 + hp, gs],
                                     qk[64:128, hp, gs], start=True, stop=True)
                for i, sp in enumerate((sp0, sp1)):
                    pme = midp.tile([GT, NG * GT], BF16, tag="pme",
                                    name=f"pme{i}")
                    nc.scalar.activation(pme[:], sp[:],
                                         mybir.ActivationFunctionType.Exp,
                                         scale=SCALE)
                    nc.vector.tensor_mul(pm[:, 2 * hp + i, :], pme[:], mask[:])

            # ---- Z, reciprocal, broadcast scale per head-pair ----
            bpss = []
            for hp in range(6):
                zp2 = ps_z.tile([2, NG * GT], F32, tag="zp", name="zp")
                nc.tensor.matmul(zp2[:], onesW[:, 0, :], pm[:, 2 * hp, :],
                                 start=True, stop=False)
                nc.tensor.matmul(zp2[:], onesW[:, 1, :], pm[:, 2 * hp + 1, :],
                                 start=False, stop=True)
                zr2 = zrp.tile([2, NG * GT], F32, tag="zr", name="zrf")
                nc.vector.reciprocal_approx_fast(zr2[:], zp2[:])
                zr2r = zrp.tile([2, NG * GT], F32R, tag="zrr", name="zrr")
                with nc.allow_low_precision(reason="f32r rounding for PE"):
                    nc.vector.tensor_copy(zr2r[:], zr2[:])
                bps = ps_big.tile([128, CHUNK], F32, tag="big", name="bps")
                nc.tensor.matmul(bps[:], ind2[:], zr2r[:],
                                 start=True, stop=True)
                bsb = bpool.tile([128, CHUNK], F32, tag="bpssb", name="bsb")
                nc.scalar.copy(bsb[:], bps[:])
                bpss.append(bsb)

            # ---- PV (quadrant pairs, d-major out) + fused normalize ----
            oT = otp.tile([128, KT, CHUNK], BF16, tag="oT")
            for hp in range(6):
                for g in range(NG):
                    gs = slice(g * GT, (g + 1) * GT)
                    pvp = ps_pv.tile([128, GT], F32, tag="pvp", name="pvp")
                    nc.tensor.matmul(pvp[0:64, :], v2[:, g, 2 * hp, :],
                                     pm[:, 2 * hp, gs], start=True, stop=True)
                    nc.tensor.matmul(pvp[64:128, :], v2[:, g, 2 * hp + 1, :],
                                     pm[:, 2 * hp + 1, gs],
                                     start=True, stop=True)
                    nc.vector.tensor_mul(oT[:, hp, gs], pvp[:],
                                         bpss[hp][:, gs])

            oT_prev = oT

        emit_out_proj(NCHUNK - 1, oT_prev)

    nc.compile()
    return nc


def _to_blocks_host(x):
    """[B_sub, 3136, d] raster -> [B_sub, ch, g, (b7 ir ic), d] block order."""
    b, n, d = x.shape
    # n = (ch, br, ir, h2, b7, ic) with sizes (7, 2, 4, 2, 7, 4)
    x = x.reshape(b, 7, 2, 4, 2, 7, 4, d)
    x = x.transpose(0, 1, 2, 4, 5, 3, 6, 7)   # b ch br h2 b7 ir ic d
    return np.ascontiguousarray(x.reshape(b, 7, NG, GT, d))


def _from_blocks_host(o):
    """[B_sub, ch, g, t, d] block order -> [B_sub, 3136, d]."""
    b = o.shape[0]
    o = o.reshape(b, 7, 2, 2, 7, 4, 4, DIM)   # b ch br h2 b7 ir ic d
    o = o.transpose(0, 1, 2, 5, 3, 4, 6, 7)   # b ch br ir h2 b7 ic d
    return np.ascontiguousarray(o.reshape(b, N, DIM))


def _prep_x_core(x_core):
    """[B_LOC, 3136, 768] fp32 -> [KT, 128, TOK] bf16 d-major block order."""
    xb = _to_blocks_host(x_core)                     # [2, 7, 4, 112, 768]
    xt = xb.reshape(TOK, DIM).T                      # [768, 6272]
    return np.ascontiguousarray(xt.reshape(KT, 128, TOK)).astype(NPBF16)


def _unprep_o_core(o6):
    """[KT, 128, TOK] bf16 d-major -> [B_LOC, 3136, 768] fp32."""
    o = np.asarray(o6, dtype=np.float32).reshape(DIM, TOK).T  # [6272, 768]
    o = np.ascontiguousarray(o).reshape(B_LOC, 7, NG, GT, DIM)
    return _from_blocks_host(o)


def make_in_maps(x, w_qkv, w_out, b_out):
    x = np.ascontiguousarray(x, dtype=np.float32)
    wq = np.ascontiguousarray(w_qkv, dtype=np.float32).astype(NPBF16)
    wo = np.ascontiguousarray(w_out, dtype=np.float32).astype(NPBF16)
    bo = np.ascontiguousarray(b_out, dtype=np.float32)
    return [
        {"x": _prep_x_core(x[c * B_LOC:(c + 1) * B_LOC]),
         "w_qkv": wq, "w_out": wo, "b_out": bo}
        for c in range(NCORES)
    ]


def kernel(x, w_qkv, w_out, b_out):
    if "nc" not in _CACHE:
        _CACHE["nc"] = _build()
    nc = _CACHE["nc"]

    in_maps = make_in_maps(x, w_qkv, w_out, b_out)
    res = run_bass_kernel_spmd(nc, in_maps, core_ids=list(range(NCORES)))
    out = np.concatenate(
        [_unprep_o_core(res.results[c]["o"]) for c in range(NCORES)], axis=0)
    return out.astype(np.float32)
